# revision 12
# baseline (speedup 1.0000x reference)
"""Trainium2 Bass kernel for CumsumAttention (v3).

Full-input contract: kernel(**inputs) takes the complete (unsharded) inputs
and returns the full [B, T, C] float32 output. Internally the work is
data-parallel over the batch dimension across 8 NeuronCores (2 batches per
core); each core runs the same Bass/Tile program on its own batch shard.

Math (per batch, reference semantics):
  k = x @ Wk.T ; q = x @ Wq.T ; v = x @ Wv.T   (biases all zero here)
  angle[t] = sum_{s>=t} ang_emb[idx[s]]        (reverse cumsum over T)
  rot/inv_rot: per channel-pair rotation by angle
  wei  = softplus((rot(k) @ rot(q).T) / sqrt(C)) masked causally (s <= t)
  out  = inv_rot(wei @ rot(v)) @ Wp.T

Engine plan (vs the 337.5us fp16 baseline):
  - Logits matmul in fp8e4 DoubleRow (2 K-tiles per instruction, 2x MAC
    rate); rot(k)/rot(q) written to fp8 by the DVE rotation.  Measured
    rel err ~1.3e-2 < 2e-2.
  - v-projection accumulates ci-outer across PSUM-resident groups so
    compute starts on the first x/Wv chunk; the angle-cumsum PE chain is
    interleaved one tile per chunk so its carry round-trips hide.
  - PSUM evacuation for the v-projection and attention-output goes to
    GpSimd, keeping ACT free for the serial Exp/Ln softplus chain that
    gates phase D; phase D consumes softplus slices in completion order
    via sj-split waves.
  - Causal masking runs in exp-space between Exp and Ln (ln(0+1)=0), so
    diagonal-block masks overlap the off-diagonal Ln chain.
  - DMA: one descriptor only sustains ~50GB/s and costs ~0.6us of issue
    on its engine, so loads are chunked across parallel queues and issued
    from sync/scalar/gpsimd; batch b+1's x/Wv/ang/Wk are prefetched
    during batch b's attention phases at points where the issuing engine
    is idle.
  - y is stored fp16 (2MB/batch) and upcast on host.
"""

import sys
import types
from contextlib import ExitStack

import numpy as np

if "/opt/trn_rl_repo" not in sys.path:
    sys.path.insert(0, "/opt/trn_rl_repo")

B, T, C = 16, 1024, 1024
D = C // 2
NCORES = 8
BPC = B // NCORES          # batches per core
P = 128                    # partitions
NT = T // P                # t tiles
NCH = C // P               # c tiles
ND = D // P                # d tiles (channel pairs)
H = 512                    # matmul free-dim block
PI = float(np.pi)
SCALE = float(C ** -0.5)

_CACHE = {}


def _install_profile_hook():
    """Register the axon NTFF profile hook if the image's antenv lacks it."""
    try:
        import antenv
        from trn_agent_boot.trn_boot import _ntff_profile_via_ctypes
    except Exception:
        return
    if "antenv.axon_hooks" in sys.modules:
        return
    try:
        hook = _ntff_profile_via_ctypes("/opt/axon/libaxon_pjrt.so")
    except Exception:
        return
    mod = types.ModuleType("antenv.axon_hooks")
    mod.get_axon_ntff_profile_hook = lambda: hook
    mod.set_axon_ntff_profile_hook = lambda h: None
    sys.modules["antenv.axon_hooks"] = mod
    antenv.axon_hooks = mod


def _build(has_bias: bool):
    import concourse.bass as bass  # noqa: F401
    import concourse.mybir as mybir
    import concourse.tile as tile
    from concourse import bacc
    from concourse.masks import make_identity

    dt = mybir.dt
    AF = mybir.ActivationFunctionType
    f16 = dt.float16
    f32 = dt.float32
    f8 = dt.float8e4
    DR = mybir.MatmulPerfMode.DoubleRow

    # Keep Exp/Ln in one table set and Sin in trig_and_small so the program
    # avoids mid-phase ACT table switches (a switch costs ~1.3us).
    import concourse.hw_specs as _hw_specs
    if not hasattr(_hw_specs, "_orig_get_activation_tables"):
        _hw_specs._orig_get_activation_tables = _hw_specs.get_activation_tables

        def _filtered_tables(arch):
            tabs = _hw_specs._orig_get_activation_tables(arch)
            for name, fns in tabs.items():
                if name != "natural_log_exp_and_others":
                    fns.discard(AF.Exp)
                    fns.discard(AF.Ln)
                if name != "trig_and_small":
                    fns.discard(AF.Sin)
            return tabs

        _hw_specs.get_activation_tables = _filtered_tables
        bacc.get_activation_tables = _filtered_tables

    nc = bacc.Bacc("TRN2", target_bir_lowering=False, debug=False,
                   num_devices=NCORES)

    xT_d = nc.dram_tensor("xT", [BPC, P, NCH, T], f16, kind="ExternalInput").ap()
    x8_d = nc.dram_tensor("x8", [BPC, P, NCH, T], f8, kind="ExternalInput").ap()
    ang_d = nc.dram_tensor("ang", [BPC, P, NT, D], f16, kind="ExternalInput").ap()
    wk_d = nc.dram_tensor("wk8T", [P, NCH, C], f8, kind="ExternalInput").ap()
    wq_d = nc.dram_tensor("wq8T", [P, NCH, C], f8, kind="ExternalInput").ap()
    wv_d = nc.dram_tensor("wvT", [P, NCH, C], f16, kind="ExternalInput").ap()
    wp_d = nc.dram_tensor("wpT", [P, NCH, C], f16, kind="ExternalInput").ap()
    tril_d = nc.dram_tensor("tril", [P, P], f16, kind="ExternalInput").ap()
    triu_d = nc.dram_tensor("triu", [P, P], f16, kind="ExternalInput").ap()
    if has_bias:
        bias_d = nc.dram_tensor("biases", [4, 1, C], f16, kind="ExternalInput").ap()
    y_d = nc.dram_tensor("y", [BPC, T, C], f16, kind="ExternalOutput").ap()

    with tile.TileContext(nc) as tc, ExitStack() as ctx:
        const = ctx.enter_context(tc.tile_pool(name="const", bufs=1))
        wv_pool = ctx.enter_context(tc.tile_pool(name="wvpool", bufs=1))
        wpr_pool = ctx.enter_context(tc.tile_pool(name="wprpool", bufs=1))
        w8_pool = ctx.enter_context(tc.tile_pool(name="w8pool", bufs=2))
        x8_pool = ctx.enter_context(tc.tile_pool(name="x8pool", bufs=1))
        xo_pool = ctx.enter_context(tc.tile_pool(name="xopool", bufs=2))
        a_pool = ctx.enter_context(tc.tile_pool(name="apool", bufs=1))
        st_pool = ctx.enter_context(tc.tile_pool(name="stpool", bufs=1))
        ct_pool = ctx.enter_context(tc.tile_pool(name="ctpool", bufs=1))
        sc_pool = ctx.enter_context(tc.tile_pool(name="scpool", bufs=1))
        cc_pool = ctx.enter_context(tc.tile_pool(name="ccpool", bufs=1))
        k_pool = ctx.enter_context(tc.tile_pool(name="kpool", bufs=1))
        q_pool = ctx.enter_context(tc.tile_pool(name="qpool", bufs=1))
        v_pool = ctx.enter_context(tc.tile_pool(name="vpool", bufs=1))
        spw_pool = ctx.enter_context(tc.tile_pool(name="spwpool", bufs=1))
        m_pool = ctx.enter_context(tc.tile_pool(name="mpool", bufs=2))
        sp_pool = ctx.enter_context(tc.tile_pool(name="sppool", bufs=2))
        y_pool = ctx.enter_context(tc.tile_pool(name="ypool", bufs=2))
        pmm = ctx.enter_context(tc.tile_pool(name="pmm", bufs=6, space="PSUM"))
        ptr = ctx.enter_context(tc.tile_pool(name="ptr", bufs=2, space="PSUM"))

        # ---- batch-0 input DMA first: engines are idle, land ASAP ----
        def dma_x(b):
            x_all = xo_pool.tile([P, NCH, T], f16, tag="xo")
            for th in range(2):
                for ci in range(NCH):
                    nc.sync.dma_start(x_all[:, ci, th * H:(th + 1) * H],
                                      xT_d[b, :, ci, th * H:(th + 1) * H])
            return x_all

        def dma_x8(b, engine):
            x8 = x8_pool.tile([P, NCH, T], f8, tag="x8")
            for cj in range(4):
                engine.dma_start(x8[:, 2 * cj:2 * cj + 2],
                                 x8_d[b, :, 2 * cj:2 * cj + 2])
            return x8

        def dma_w8(engine):
            wk8 = w8_pool.tile([P, NCH, C], f8, tag="w8")
            wq8 = w8_pool.tile([P, NCH, C], f8, tag="w8")
            for cj in range(4):
                engine.dma_start(wk8[:, 2 * cj:2 * cj + 2],
                                 wk_d[:, 2 * cj:2 * cj + 2])
            for cj in range(4):
                engine.dma_start(wq8[:, 2 * cj:2 * cj + 2],
                                 wq_d[:, 2 * cj:2 * cj + 2])
            return wk8, wq8

        def dma_wv_ang(b, engine):
            wv = wv_pool.tile([P, NCH, C], f16, tag="wv")
            for ci in range(NCH):
                engine.dma_start(wv[:, ci], wv_d[:, ci])
            a_all = a_pool.tile([P, NT, D], f16, tag="a")
            for g in reversed(range(4)):
                engine.dma_start(a_all[:, 2 * g:2 * g + 2],
                                 ang_d[b, :, 2 * g:2 * g + 2])
            return wv, a_all

        def dma_w(w_dram, engine):
            w_sb = wpr_pool.tile([P, NCH, C], f16, tag="w")
            for ci in range(NCH):
                engine.dma_start(w_sb[:, ci], w_dram[:, ci])
            return w_sb

        class S:
            pass

        states = [S() for _ in range(BPC)]
        st0 = states[0]
        st0.x_all = dma_x(0)
        st0.wv, st0.a_all = dma_wv_ang(0, nc.gpsimd)

        # consts after the hot DMAs (identity is only needed ~45us in)
        ident = const.tile([P, P], f16)
        make_identity(nc, ident[:])
        tril = const.tile([P, P], f16)
        triu = const.tile([P, P], f16)
        nc.scalar.dma_start(tril[:], tril_d[:])
        nc.scalar.dma_start(triu[:], triu_d[:])
        ones1f = const.tile([1, P], f32)
        nc.gpsimd.memset(ones1f[:], 1.0)
        ones1r = const.tile([1, P], dt.float32r)
        nc.scalar.activation(ones1r[:], ones1f[:], AF.Copy)
        if has_bias:
            ones_row = const.tile([1, H], f16)
            nc.gpsimd.memset(ones_row[:], 1.0)
            brows = const.tile([1, 4, C], f16)
            for i in range(4):
                nc.scalar.dma_start(brows[:, i], bias_d[i])

        def emit_vproj_and_phaseA(b, st, wk_hook=None):
            x_all, wv, a_all = st.x_all, st.wv, st.a_all
            v_all = v_pool.tile([P, NT, C], f16, tag="v")
            st.v_all = v_all

            # phase-A state threaded through the interleaved emission
            sin_t = st_pool.tile([P, NT, D], f16, tag="sin_t")
            cos_t = ct_pool.tile([P, NT, D], f16, tag="cos_t")
            st.sin_t, st.cos_t = sin_t, cos_t
            aa = {"carry": None}

            def emit_cumsum_tile(ti):
                ps = pmm.tile([P, D], f32, tag="pmm", name=f"psa{ti}")
                nc.tensor.matmul(ps[:], tril[:], a_all[:, ti],
                                 start=True, stop=(ti == NT - 1))
                if aa["carry"] is not None:
                    nc.tensor.matmul(ps[:], ones1r[:], aa["carry"][:],
                                     start=False, stop=True)
                if ti > 0:
                    carry = sp_pool.tile([1, D], dt.float32r, tag="carry",
                                         bufs=1)
                    nc.scalar.activation(carry[:], ps[0:1, :], AF.Copy)
                    aa["carry"] = carry
                u = sp_pool.tile([P, D], f16, tag="sp")
                w = sp_pool.tile([P, D], f16, tag="sp")
                nc.vector.add_range_wrap(u[:], ps[:], 0.0, PI, 2 * PI)
                nc.vector.add_range_wrap(w[:], ps[:], PI / 2, PI, 2 * PI)
                nc.scalar.activation(sin_t[:, ti], u[:], AF.Sin)
                nc.scalar.activation(cos_t[:, ti], w[:], AF.Sin)

            def vpass(tis, hooks=None):
                pss = {}
                for ti in tis:
                    for ch in range(2):
                        pss[(ti, ch)] = pmm.tile([P, H], f32, tag="pmm",
                                                 name=f"psv{ti}_{ch}")
                for ci in range(NCH):
                    for ti in tis:
                        for ch in range(2):
                            nc.tensor.matmul(
                                pss[(ti, ch)][:],
                                x_all[:, ci, ti * P:(ti + 1) * P],
                                wv[:, ci, ch * H:(ch + 1) * H],
                                start=(ci == 0),
                                stop=(ci == NCH - 1 and not has_bias))
                    if hooks and ci < len(hooks):
                        hooks[ci]()
                for ti in tis:
                    for ch in range(2):
                        ps = pss[(ti, ch)]
                        if has_bias:
                            nc.tensor.matmul(ps[:], ones_row[:, :P],
                                             brows[:, 2, ch * H:(ch + 1) * H],
                                             start=False, stop=True)
                        nc.vector.tensor_copy(v_all[:, ti, ch * H:(ch + 1) * H],
                                              ps[:])

            vpass((0, 1))
            vpass((2, 3))
            # wk lands here in the gpsimd stream: after the early v-pass
            # evacuations (so psum recycling isn't head-blocked) and well
            # before the k-projection consumes it
            if wk_hook:
                wk_hook()
            # the reverse cumsum chain rides along vpass(4,5): one tile per
            # x-chunk slot so each carry round-trip hides under 4 matmuls
            vpass((4, 5), hooks=[lambda ti=ti: emit_cumsum_tile(ti)
                                 for ti in reversed(range(NT))])
            vpass((6, 7))

            # transpose cos/sin to channel-major; th=1 groups first since the
            # reversed cumsum finishes sin/cos for ti=4..7 earliest
            sin_c = sc_pool.tile([P, ND, T], f16, tag="sin_c")
            cos_c = cc_pool.tile([P, ND, T], f16, tag="cos_c")
            st.sin_c, st.cos_c = sin_c, cos_c
            for th in (1, 0):
                for src, dst in ((cos_t, cos_c), (sin_t, sin_c)):
                    for dj in range(ND):
                        pst = ptr.tile([P, H], f16, tag="ptr")
                        for tq in range(4):
                            ti = th * 4 + tq
                            nc.tensor.transpose(pst[:, tq * P:(tq + 1) * P],
                                                src[:, ti, dj * P:(dj + 1) * P],
                                                ident[:])
                        nc.vector.tensor_copy(dst[:, dj, th * H:(th + 1) * H],
                                              pst[:])

            # rotate v in place (t-major)
            for ti in range(NT):
                z0 = v_all[:, ti, 0:D]
                z1 = v_all[:, ti, D:C]
                ma = m_pool.tile([P, T], f16, tag="ma", bufs=1)
                mb = m_pool.tile([P, T], f16, tag="mb", bufs=1)
                mc = m_pool.tile([P, T], f16, tag="mc", bufs=1)
                nc.vector.tensor_mul(ma[:, 0:D], z0, cos_t[:, ti])
                nc.vector.tensor_mul(mb[:, 0:D], z1, sin_t[:, ti])
                nc.vector.tensor_mul(mc[:, 0:D], z0, sin_t[:, ti])
                nc.vector.tensor_sub(z0, ma[:, 0:D], mb[:, 0:D])
                nc.vector.tensor_mul(ma[:, D:C], z1, cos_t[:, ti])
                nc.vector.tensor_add(z1, mc[:, 0:D], ma[:, D:C])

        def emit_proj(b, st, w8, out_pool, tag, bias_idx):
            """k/q projection in c-major via fp8 DoubleRow (weights carry a
            x32 prescale; folded out at the Exp).  Rotation output f16."""
            x8 = st.x8
            cos_c, sin_c = st.cos_c, st.sin_c
            out16 = out_pool.tile([P, NCH, T], f16, tag=tag)
            for p in range(ND):
                z0 = m_pool.tile([P, T], f16, tag="z0", bufs=1)
                z1 = m_pool.tile([P, T], f16, tag="z1", bufs=1)
                for th in range(2):
                    for zi, co in ((0, p), (1, p + 4)):
                        z = z0 if zi == 0 else z1
                        ps = pmm.tile([P, H], f32, tag="pmm")
                        for cj in range(NCH // 2):
                            nc.tensor.matmul(ps[:],
                                             w8[:, 2 * cj:2 * cj + 2,
                                                co * P:(co + 1) * P],
                                             x8[:, 2 * cj:2 * cj + 2,
                                                th * H:(th + 1) * H],
                                             start=(cj == 0),
                                             stop=(cj == NCH // 2 - 1
                                                   and not has_bias),
                                             perf_mode=DR)
                        if has_bias:
                            nc.tensor.matmul(ps[:],
                                             brows[:, bias_idx, co * P:(co + 1) * P],
                                             ones_row[:], start=False, stop=True)
                        nc.scalar.activation(z[:, th * H:(th + 1) * H], ps[:],
                                             AF.Copy)
                    hs = slice(th * H, (th + 1) * H)
                    cs = cos_c[:, p, hs]
                    sn = sin_c[:, p, hs]
                    ma = m_pool.tile([P, H], f16, tag="ma", bufs=1)
                    mb = m_pool.tile([P, H], f16, tag="mb", bufs=1)
                    mc = m_pool.tile([P, H], f16, tag="mc", bufs=1)
                    nc.vector.tensor_mul(ma[:], z0[:, hs], cs)
                    nc.vector.tensor_mul(mb[:], z1[:, hs], sn)
                    nc.vector.tensor_mul(mc[:], z0[:, hs], sn)
                    nc.vector.tensor_sub(out16[:, p, hs], ma[:], mb[:])
                    nc.vector.tensor_mul(ma[:], z1[:, hs], cs)
                    nc.vector.tensor_add(out16[:, p + 4, hs], mc[:], ma[:])
            return out16

        def emit_phaseC(b, st):
            """wei^T = softplus(q.k / sqrt(C)): f16 logits, Exp,
            exp-space causal mask, Ln.  The x32 weight prescale on both
            k and q is folded out via the Exp scale (1/1024)."""
            k16, q16 = st.k16, st.q16
            spw = spw_pool.tile([P, 12, H], f16, tag="spw")
            st.spw = spw
            for th in range(2):
                smax = 4 * th + 3
                for si in range(smax + 1):
                    off = max(0, si * P - th * H)
                    ncols = H - off
                    col0 = th * H + off
                    ps = pmm.tile([P, H], f32, tag="pmm")
                    for j, ci in enumerate((0, 4, 2, 6, 1, 5, 3, 7)):
                        nc.tensor.matmul(ps[:, 0:ncols],
                                         q16[:, ci, si * P:(si + 1) * P],
                                         k16[:, ci, col0:col0 + ncols],
                                         start=(j == 0), stop=(j == NCH - 1))
                    nc.scalar.activation(spw[:, 4 * th + si, off:off + ncols],
                                         ps[:, 0:ncols], AF.Exp,
                                         scale=SCALE / 1024.0)
                # diagonal blocks: mask in exp-space (DVE, overlaps ACT),
                # then Ln everything; ln(0 + 1) = 0 reproduces the mask.
                for si in range(4 * th, 4 * th + 4):
                    off = si * P - th * H
                    sl = 4 * th + si
                    nc.vector.tensor_mul(spw[:, sl, off:off + P],
                                         spw[:, sl, off:off + P], triu[:])
                for si in range(smax + 1):
                    off = max(0, si * P - th * H)
                    sl = 4 * th + si
                    nc.scalar.activation(spw[:, sl, off:H],
                                         spw[:, sl, off:H],
                                         AF.Ln, bias=1.0)

        def emit_phaseD(b, st):
            """out^T = v.T @ wei^T, inverse-rotated -> ro.  sj-split waves:
            each psum group first accumulates the early softplus slices so
            the tail of the ACT chain is only needed late."""
            v_all, spw = st.v_all, st.spw
            cos_c, sin_c = st.cos_c, st.sin_c
            ro = xo_pool.tile([P, NCH, T], f16, tag="xo")
            st.ro = ro

            def group(th, pps, waves):
                smax = 4 * th + 3
                pss = {}
                for pp in pps:
                    pss[(pp, 0)] = pmm.tile([P, H], f32, tag="pmm",
                                            name=f"psd{pp}_0")
                    pss[(pp, 1)] = pmm.tile([P, H], f32, tag="pmm",
                                            name=f"psd{pp}_1")
                for wave in waves:
                    for pp in pps:
                        for zi, pq in ((0, pp), (1, pp + 4)):
                            ps = pss[(pp, zi)]
                            for sj in wave:
                                off = max(0, sj * P - th * H)
                                nc.tensor.matmul(
                                    ps[:, off:H],
                                    v_all[:, sj, pq * P:(pq + 1) * P],
                                    spw[:, 4 * th + sj, off:H],
                                    start=(sj == 0), stop=(sj == smax))
                for pp in pps:
                    hs = slice(th * H, (th + 1) * H)
                    cs = cos_c[:, pp, hs]
                    sn = sin_c[:, pp, hs]
                    oz = m_pool.tile([P, T], f16, tag="oz", bufs=1)
                    nc.vector.tensor_copy(oz[:, 0:H], pss[(pp, 0)][:])
                    nc.vector.tensor_copy(oz[:, H:T], pss[(pp, 1)][:])
                    # split the inverse rotation across gpsimd (SBUF-only
                    # ops allowed there) and DVE so neither gates phase D
                    ma = m_pool.tile([P, T], f16, tag="ma", bufs=1)
                    mb = m_pool.tile([P, T], f16, tag="mb", bufs=1)
                    nc.gpsimd.tensor_mul(ma[:, 0:H], oz[:, 0:H], cs)
                    nc.gpsimd.tensor_mul(mb[:, 0:H], oz[:, H:T], sn)
                    nc.gpsimd.tensor_add(ro[:, pp, hs], ma[:, 0:H], mb[:, 0:H])
                    nc.vector.tensor_mul(ma[:, H:T], oz[:, 0:H], sn)
                    nc.vector.tensor_mul(mb[:, H:T], oz[:, H:T], cs)
                    nc.vector.tensor_sub(ro[:, pp + 4, hs], mb[:, H:T],
                                         ma[:, H:T])

            group(0, (0, 1, 2), ((0, 1), (2, 3)))
            group(0, (3,), ((0, 1), (2, 3)))
            group(1, (0, 1, 2), ((0, 1, 2, 3), (4, 5, 6, 7)))
            group(1, (3,), ((0, 1, 2, 3), (4, 5, 6, 7)))

        def emit_phaseE(b, st, wp):
            ro = st.ro
            ci_order = [0, 1, 2, 4, 5, 6, 3, 7]
            for ti in range(NT):
                for ch in range(2):
                    yt = y_pool.tile([P, H], f16, tag="y")
                    ps = pmm.tile([P, H], f32, tag="pmm")
                    for j, ci in enumerate(ci_order):
                        nc.tensor.matmul(ps[:], ro[:, ci, ti * P:(ti + 1) * P],
                                         wp[:, ci, ch * H:(ch + 1) * H],
                                         start=(j == 0),
                                         stop=(j == NCH - 1 and not has_bias))
                    if has_bias:
                        nc.tensor.matmul(ps[:], ones_row[:, :P],
                                         brows[:, 3, ch * H:(ch + 1) * H],
                                         start=False, stop=True)
                    nc.scalar.activation(yt[:], ps[:], AF.Copy)
                    nc.sync.dma_start(
                        y_d[b, ti * P:(ti + 1) * P, ch * H:(ch + 1) * H],
                        yt[:])

        # ================= schedule =================
        for b in range(BPC):
            st = states[b]
            wk_box = {}

            def wk_hook():
                wk_box["w8"] = dma_w8(nc.gpsimd)
                st.x8 = dma_x8(b, nc.scalar)

            emit_vproj_and_phaseA(b, st, wk_hook=wk_hook)
            # prefetch next batch's x/wv/ang while PE chews on k-proj;
            # sync and gpsimd queues are otherwise idle here
            if b + 1 < BPC:
                nxt = states[b + 1]
                nxt.x_all = dma_x(b + 1)
            wk8, wq8 = wk_box["w8"]
            st.k16 = emit_proj(b, st, wk8, k_pool, "k", 0)
            if b + 1 < BPC:
                nxt.wv, nxt.a_all = dma_wv_ang(b + 1, nc.gpsimd)
            st.q16 = emit_proj(b, st, wq8, q_pool, "q", 1)
            emit_phaseC(b, st)
            emit_phaseD(b, st)
            wp = dma_w(wp_d, nc.gpsimd)
            emit_phaseE(b, st, wp)

    nc.compile()
    return nc


def _get_program(has_bias: bool):
    key = ("prog3", has_bias)
    if key not in _CACHE:
        _CACHE[key] = _build(has_bias)
    return _CACHE[key]


def _prep_host(x, idx, Wk, Wq, Wv, Wp, ang_emb, biases):
    import ml_dtypes
    e4 = ml_dtypes.float8_e4m3
    perm = np.concatenate([np.arange(0, C, 2), np.arange(1, C, 2)])
    # x: [B, T, C] -> per batch [P, NCH, T] (partition-major chunks of x^T)
    xT = np.transpose(np.asarray(x, np.float32), (0, 2, 1))      # [B, C, T]
    xTt = xT.reshape(B, NCH, P, T)
    xTt = np.ascontiguousarray(np.transpose(xTt, (0, 2, 1, 3)))
    xT16 = xTt.astype(np.float16).reshape(NCORES, BPC, P, NCH, T)
    xT8 = xTt.astype(e4).reshape(NCORES, BPC, P, NCH, T)
    idx = np.asarray(idx).astype(np.int64)
    ang = np.asarray(ang_emb, np.float32)[idx]                   # [B, T, D]
    ang16 = ang.astype(np.float16).reshape(B, NT, P, D)
    ang16 = np.ascontiguousarray(np.transpose(ang16, (0, 2, 1, 3)))
    ang16 = ang16.reshape(NCORES, BPC, P, NT, D)

    def wtile(m, dtype=np.float16):
        w = np.ascontiguousarray(m).astype(dtype).reshape(NCH, P, C)
        return np.ascontiguousarray(np.transpose(w, (1, 0, 2)))

    # k/q weights carry x32 so fp8e4 values sit in the normal range;
    # folded out by the Exp scale (1/1024) in phase C.
    wk8T = wtile(np.asarray(Wk, np.float32)[perm].T * 32.0, e4)
    wq8T = wtile(np.asarray(Wq, np.float32)[perm].T * 32.0, e4)
    wvT = wtile(np.asarray(Wv, np.float32)[perm].T)
    wpT = wtile(np.asarray(Wp, np.float32)[:, perm].T)

    tril = np.tril(np.ones((P, P), np.float16))
    triu = np.triu(np.ones((P, P), np.float16))

    consts = dict(wk8T=wk8T, wq8T=wq8T, wvT=wvT, wpT=wpT, tril=tril, triu=triu)
    bk, bq, bv, bp = (np.asarray(b_, np.float32) for b_ in biases)
    has_bias = any(np.any(b_ != 0) for b_ in (bk, bq, bv, bp))
    if has_bias:
        brows = np.stack([bk[perm] * 32.0, bq[perm] * 32.0, bv[perm],
                          bp]).astype(np.float16)
        consts["biases"] = brows.reshape(4, 1, C)
    return xT16, xT8, ang16, consts, has_bias


def run_on_device(x, idx, Wk, Wq, Wv, Wp, ang_emb, biases, trace=False):
    _install_profile_hook()
    import concourse.bass_utils as bass_utils
    bass_utils.upload_artifacts = lambda tmpdir: "local://" + tmpdir
    from concourse.bass_utils import run_bass_kernel_spmd

    xT16, xT8, ang16, consts, has_bias = _prep_host(x, idx, Wk, Wq, Wv, Wp,
                                                    ang_emb, biases)
    nc = _get_program(has_bias)
    in_maps = []
    for c in range(NCORES):
        m = {"xT": xT16[c], "x8": xT8[c], "ang": ang16[c]}
        m.update(consts)
        in_maps.append(m)
    res = run_bass_kernel_spmd(nc, in_maps, list(range(NCORES)), trace=trace)
    y = np.empty((B, T, C), np.float32)
    for c in range(NCORES):
        y[c * BPC:(c + 1) * BPC] = res.results[c]["y"].astype(np.float32)
    return y, res


def kernel(x, idx, Wk, bk, Wq, bq, Wv, bv, Wp, bp, ang_emb):
    y, _ = run_on_device(x, idx, Wk, Wq, Wv, Wp, ang_emb, (bk, bq, bv, bp))
    return y

